# revision 9
# baseline (speedup 1.0000x reference)
"""AugmentedGRU (ragged) Trainium2 kernel.

Strategy: data-parallel over batch B=64 across 8 cores (8 seq/core).
Phase 1 (parallel): gx = x_aug @ w1  (w1 = [w_ih.T; combined bias row]),
  tiled GEMM, result staged in DRAM.
Phase 2 (sequential scan, T=1024): per step
  gh = h @ w_hh.T   (12 matmuls, lhsT = transposed hidden state tiles)
  r,z = sigmoid(gx_rz + gh_rz) ; n = tanh(gx_n + r*(gh_n + b_hh_n))
  h' = n + z*(h - n)
  hT tiles refreshed via PE transpose (matmul with identity rhs).
Raggedness handled entirely in numpy post-processing: the on-chip scan is
unmasked; outputs past each sequence's length are zeroed on host and
h_last[b] = out[len[b]-1, b].
"""
import numpy as np

T, B, I, H = 1024, 64, 512, 512
G = 3 * H            # 1536
NC = 8
BL = B // NC         # 8 sequences per core
KA = 640             # padded contraction for phase 1 (512 + bias row + pad)


def _build(nc, mybir, tile, bass):
    f32 = mybir.dt.float32
    ds = bass.ds

    x_augT = nc.dram_tensor("x_augT", [(T * BL) // 128, 128, KA], f32,
                            kind="ExternalInput")
    w1 = nc.dram_tensor("w1", [KA, G], f32, kind="ExternalInput")
    w2 = nc.dram_tensor("w2", [4, 128, G], f32, kind="ExternalInput")
    bhn = nc.dram_tensor("bhn", [BL, H], f32, kind="ExternalInput")
    eye = nc.dram_tensor("eye", [BL, BL], f32, kind="ExternalInput")
    zeros = nc.dram_tensor("zeros", [128, H], f32, kind="ExternalInput")
    out_h = nc.dram_tensor("out_h", [T * BL, H], f32, kind="ExternalOutput")

    MT = (T * BL) // 128   # 64 m-tiles in phase 1

    with tile.TileContext(nc) as tc:
        with (
            tc.tile_pool(name="wpool", bufs=1) as wpool,
            tc.tile_pool(name="state", bufs=1) as state,
            tc.tile_pool(name="dram", bufs=1, space="DRAM") as dpool,
        ):
            gx_dram = dpool.tile([T * BL, G], f32)

            # resident weights / constants
            w1_sb = []
            for k in range(5):
                t_ = wpool.tile([128, G], f32, tag=f"w1_{k}")
                nc.gpsimd.dma_start(t_[:], w1[k * 128:(k + 1) * 128, :])
                w1_sb.append(t_)
            w2_sb = []
            for k in range(4):
                t_ = wpool.tile([128, G], f32, tag=f"w2_{k}")
                nc.gpsimd.dma_start(t_[:], w2[k, :, :])
                w2_sb.append(t_)
            bhn_sb = wpool.tile([BL, H], f32, tag="bhn")
            nc.gpsimd.dma_start(bhn_sb[:], bhn[:, :])
            eye_sb = wpool.tile([BL, BL], f32, tag="eye")
            nc.gpsimd.dma_start(eye_sb[:], eye[:, :])

            # persistent scan state
            h_cur = state.tile([BL, H], f32, tag="h_cur")
            nc.gpsimd.dma_start(h_cur[:], zeros[0:BL, :])
            hT = []
            for k in range(4):
                t_ = state.tile([128, BL], f32, tag=f"hT_{k}")
                nc.gpsimd.dma_start(t_[:], zeros[:, 0:BL])
                hT.append(t_)

            # ---------------- Phase 1: gx GEMM ----------------
            with (
                tc.tile_pool(name="xin", bufs=4) as xin,
                tc.tile_pool(name="ps1", bufs=2, space="PSUM") as ps1p,
            ):
                for m in range(MT):
                    xt = xin.tile([128, KA], f32, tag="xt")
                    nc.gpsimd.dma_start(xt[:], x_augT[m, :, :])
                    ps = ps1p.tile([128, G], f32, tag="ps1")
                    for k in range(5):
                        for n in range(3):
                            nc.tensor.matmul(
                                ps[:, n * 512:(n + 1) * 512],
                                xt[:, k * 128:(k + 1) * 128],
                                w1_sb[k][:, n * 512:(n + 1) * 512],
                                start=(k == 0), stop=(k == 4))
                    gxs = xin.tile([128, G], f32, tag="gxs")
                    nc.vector.tensor_copy(gxs[:], ps[:])
                    nc.gpsimd.dma_start(
                        gx_dram[m * 128:(m + 1) * 128, :], gxs[:])

            # ---------------- Phase 2: scan ----------------
            with (
                tc.tile_pool(name="tmp", bufs=3) as tmp,
                tc.tile_pool(name="ps2", bufs=2, space="PSUM") as ps2p,
                tc.tile_pool(name="pst", bufs=2, space="PSUM") as pstp,
            ):
                def body(row0):
                    gxt = tmp.tile([BL, G], f32, tag="gxt")
                    nc.gpsimd.dma_start(gxt[:], gx_dram[ds(row0, BL), :])
                    ps = ps2p.tile([BL, G], f32, tag="ps2")
                    for k in range(4):
                        for n in range(3):
                            nc.tensor.matmul(
                                ps[:, n * 512:(n + 1) * 512],
                                hT[k][:, 0:BL],
                                w2_sb[k][:, n * 512:(n + 1) * 512],
                                start=(k == 0), stop=(k == 3))
                    srz = tmp.tile([BL, 2 * H], f32, tag="srz")
                    nc.vector.tensor_add(srz[:], ps[:, 0:2 * H], gxt[:, 0:2 * H])
                    rz = tmp.tile([BL, 2 * H], f32, tag="rz")
                    nc.scalar.activation(
                        rz[:], srz[:], mybir.ActivationFunctionType.Sigmoid)
                    hn = tmp.tile([BL, H], f32, tag="hn")
                    nc.vector.tensor_add(hn[:], ps[:, 2 * H:G], bhn_sb[:])
                    hnr = tmp.tile([BL, H], f32, tag="hnr")
                    nc.vector.tensor_mul(hnr[:], rz[:, 0:H], hn[:])
                    sn = tmp.tile([BL, H], f32, tag="sn")
                    nc.vector.tensor_add(sn[:], hnr[:], gxt[:, 2 * H:G])
                    nt = tmp.tile([BL, H], f32, tag="nt")
                    nc.scalar.activation(
                        nt[:], sn[:], mybir.ActivationFunctionType.Tanh)
                    d = tmp.tile([BL, H], f32, tag="d")
                    nc.vector.tensor_sub(d[:], h_cur[:], nt[:])
                    e = tmp.tile([BL, H], f32, tag="e")
                    nc.vector.tensor_mul(e[:], rz[:, H:2 * H], d[:])
                    nc.vector.tensor_add(h_cur[:], nt[:], e[:])
                    nc.gpsimd.dma_start(out_h[ds(row0, BL), :], h_cur[:])
                    for k in range(4):
                        tp = pstp.tile([128, BL], f32, tag="tp")
                        nc.tensor.matmul(
                            tp[:], h_cur[:, k * 128:(k + 1) * 128],
                            eye_sb[:], start=True, stop=True)
                        nc.vector.tensor_copy(hT[k][:], tp[:])

                tc.For_i_unrolled(0, T * BL, BL, body, max_unroll=8)
    return nc


def kernel(x, batch_lengths, w_ih, w_hh, b_ih, b_hh, _trace=False):
    import concourse.bass as bass
    import concourse.mybir as mybir
    from concourse import bacc, tile
    from concourse.bass_utils import run_bass_kernel_spmd

    x = np.asarray(x, np.float32)
    lengths = np.asarray(batch_lengths).astype(np.int64)
    w_ih = np.asarray(w_ih, np.float32)
    w_hh = np.asarray(w_hh, np.float32)
    b_ih = np.asarray(b_ih, np.float32)
    b_hh = np.asarray(b_hh, np.float32)

    # host-side prep (sharding + weight packing)
    bias_combo = b_ih.copy()
    bias_combo[0:2 * H] += b_hh[0:2 * H]
    w1 = np.zeros((KA, G), np.float32)
    w1[0:I, :] = w_ih.T
    w1[I, :] = bias_combo
    w2 = np.ascontiguousarray(w_hh.T.reshape(4, 128, G))
    bhn = np.tile(b_hh[2 * H:G], (BL, 1)).astype(np.float32)
    eye = np.eye(BL, dtype=np.float32)
    zeros = np.zeros((128, H), np.float32)

    nc = bacc.Bacc()
    _build(nc, mybir, tile, bass)
    nc.compile()

    in_maps = []
    for c in range(NC):
        xc = x[:, c * BL:(c + 1) * BL, :].reshape(T * BL, I)   # row = t*BL+b
        x_aug = np.zeros((T * BL, KA), np.float32)
        x_aug[:, 0:I] = xc
        x_aug[:, I] = 1.0
        xpack = x_aug.reshape(64, 128, 5, 128).transpose(0, 3, 2, 1)
        xpack = np.ascontiguousarray(xpack.reshape(64, 128, KA))
        in_maps.append({
            "x_augT": xpack, "w1": w1, "w2": w2,
            "bhn": bhn, "eye": eye, "zeros": zeros,
        })

    try:
        res = run_bass_kernel_spmd(nc, in_maps, core_ids=list(range(NC)),
                                   trace=_trace)
    except ModuleNotFoundError:
        res = run_bass_kernel_spmd(nc, in_maps, core_ids=list(range(NC)),
                                   trace=False)
    results = res.results

    out = np.zeros((T, B, H), np.float32)
    for c in range(NC):
        out[:, c * BL:(c + 1) * BL, :] = results[c]["out_h"].reshape(T, BL, H)
    h_last = out[lengths - 1, np.arange(B)].copy()
    mask = np.arange(T)[:, None] >= lengths[None, :]
    out[mask] = 0.0
    if _trace:
        kernel._last_exec_ns = res.exec_time_ns
    return out, h_last[None]
